# revision 63
# baseline (speedup 1.0000x reference)
"""BasicLS on 8 trn2 cores — reversed-matmul stats, batch-major solve.

Host staging (layout/precision choice): cast x to fp16 and pre-swizzle per
core into tiles [128=(32g+m), (d, q, p)] so the kernel needs no on-chip cast
or input transposes, and input DMA bytes halve.

Per 4096-batch tile t (batch b = t*4096 + p*32 + 4q + g):
  1. DMA xt [128=(32g+m), (d4, q8, p128)] fp16 (8KB/partition, full rate);
     the first two tiles split per q-half so compute starts earlier.
  2. Products: batched cross muls on DVE (pairs (01,12,23) in one 3-plane op,
     (02,13) in one 2-plane op), pair (03) on Pool, 3 squares in one ACT op.
  3. Stats via reversed matmuls: for each of 13 stat planes and each q, a
     matmul with lhsT = the plane's [128, 128] data block (as the weights;
     the weight-load streams the same columns a forward matmul would, so
     this is cost-neutral on the PE) and rhs = W [128, 4] (W[32g+m, g] =
     1/32) yields out[p-col, g] = the group-g m-sum — batch-major stats
     straight into PSUM [128, q, s, g], no transpose-back stage. The 1/32
     scale keeps all solve quantities O(1) so fp16 temporaries are safe and
     the 4x4 pivot is 1.
  4. ACT copy PSUM -> per-chunk ST [128, ct, 8, 52] fp16 (52 = 4s+g, stat
     blocks ordered T0..T3 | S11,S22,S33 | S12,S13,S23 | S01,S02,S03 so the
     solve reads contiguous 3-stat slices).
  5. Solve per chunk (generator, pumped between tiles round-robin): Schur-
     eliminate the unit pivot (diag products dd,gg,ii as one ACT square),
     then symmetric 3x3 adjugate solve with slot-packed SCH6=[b,c,f,a,e,h]
     so the 12 adjugate products batch into 4 two-wide ops + 1 narrow + one
     ACT square (b2,c2,f2); fp16 temps, fp32 det + fast-approx reciprocal.
     Wide ops on DVE (fp16 2x), narrow ops on Pool (DVE mixed in for the
     post-stream tail chunk).
  6. Output DMA per chunk on the ACT engine's HWDGE (keeps the SP queue
     free for input DMAs).
"""

import itertools
import os as _os

import numpy as np

import concourse.bacc as bacc
import concourse.tile as tile
from concourse import mybir
from concourse.bass import AP as BassAP
from concourse.bass_utils import run_bass_kernel_spmd

F32 = mybir.dt.float32
F16 = mybir.dt.float16

B, M, D = 262144, 32, 4
NCORES = 8
BC = B // NCORES          # 32768
NT = 8
TB = BC // NT             # 4096
NQ, NG = 8, 4
IVN = 1.0 / 32.0          # stat scaling (weights hold 1/32)

# stat order: 0..3 = T0..T3; 4 S11, 5 S22, 6 S33; 7 S12, 8 S13, 9 S23;
# 10 S01, 11 S02, 12 S03  (diag block / cross block / r-block: lets the
# solve use contiguous 3-stat slices and ACT squares)
# product slots in PRA: 0..2 cross (01,12,23), 3..4 cross (02,13), 5 cross
# (03), 6..8 squares (11,22,33)
STAT_RHS = {10: 0, 7: 1, 9: 2, 11: 3, 8: 4, 12: 5, 4: 6, 5: 7, 6: 8}
# emission order: T stats (xt only), then products as they land
STAT_ORDER = [0, 1, 2, 3, 10, 7, 9, 11, 8, 4, 5, 6, 12]
NS = 13

# (start tile, n tiles) per solve chunk; sweepable via KB_CHUNKS
_sizes = [int(x) for x in
          _os.environ.get("KB_CHUNKS", "3,3,2").split(",")]
assert sum(_sizes) == NT
CHUNKS = []
for _sz in _sizes:
    CHUNKS.append((sum(s for _, s in CHUNKS), _sz))
# yield-groups of pending solves to emit after each tile's stats
PUMP_GROUPS = {
    int(k): int(v)
    for k, v in (kv.split(":") for kv in
                 _os.environ.get("KB_PUMP",
                                 "2:1,3:2,4:2,5:3,6:4,7:99").split(","))
}


def _emit(nc, tc, xd, yd):
    V, G, A = nc.vector, nc.gpsimd, nc.scalar

    x_all = xd.ap()                                   # [NT, 128, 4096]
    y_all = yd.ap().rearrange("(t p c) d -> p t c d", t=NT, p=128)

    with (
        tc.tile_pool(name="const", bufs=1) as cpool,
        tc.tile_pool(name="xin", bufs=int(_os.environ.get("KB_XBUFS", "3"))) as xpool,
        tc.tile_pool(name="pr", bufs=int(_os.environ.get("KB_PBUFS", "2"))) as prpool,
        tc.tile_pool(name="stat", bufs=1) as spool,
        tc.tile_pool(name="solve", bufs=1) as lpool,
        tc.tile_pool(name="psw", bufs=2, space="PSUM") as wpool,
    ):
        # reversed-matmul rhs: W[32g+m, g] = 1/32
        W = cpool.tile([128, NG], F16, name="W")
        V.memset(W, 0.0)
        for g in range(NG):
            V.memset(W[32 * g:32 * (g + 1), g:g + 1], IVN)

        # per-chunk batch-major stats [128, ct, NQ, 52] fp16
        STc = [
            spool.tile([128, ct, NQ, 52], F16, name=f"ST_{ci}", tag=f"ST_{ci}")
            for ci, (t0, ct) in enumerate(CHUNKS)
        ]

        fronts = {}

        # q's of the (03) plane on Pool (rest on DVE) — DVE/Pool balance knob
        P03Q = int(_os.environ.get("KB_P03Q", "8"))
        # q's of the (02,13) 2-plane op moved to Pool
        P2Q = int(_os.environ.get("KB_P2Q", "1"))

        def emit_products(t, xt, PRA, qs):
            """products for q-slice qs of tile t."""
            # batched cross muls: (01,12,23) one op, (02,13) one op on DVE
            V.tensor_mul(out=PRA[:, 0:3, qs, :], in0=xt[:, 0:3, qs, :],
                         in1=xt[:, 1:4, qs, :])
            lo2, hi2 = qs.start or 0, qs.stop if qs.stop is not None else NQ
            g2 = min(hi2, P2Q)
            if g2 > lo2:
                G.tensor_mul(out=PRA[:, 3:5, lo2:g2, :],
                             in0=xt[:, 0:2, lo2:g2, :],
                             in1=xt[:, 2:4, lo2:g2, :])
            if hi2 > max(lo2, g2):
                v2 = max(lo2, g2)
                V.tensor_mul(out=PRA[:, 3:5, v2:hi2, :],
                             in0=xt[:, 0:2, v2:hi2, :],
                             in1=xt[:, 2:4, v2:hi2, :])
            # (03) split between Pool (first P03Q q's) and DVE
            lo, hi = qs.start or 0, qs.stop if qs.stop is not None else NQ
            gq = min(hi, P03Q)
            if gq > lo:
                G.tensor_mul(out=PRA[:, 5, lo:gq, :], in0=xt[:, 0, lo:gq, :],
                             in1=xt[:, 3, lo:gq, :])
            if hi > max(lo, gq):
                vlo = max(lo, gq)
                V.tensor_mul(out=PRA[:, 5, vlo:hi, :], in0=xt[:, 0, vlo:hi, :],
                             in1=xt[:, 3, vlo:hi, :])
            # squares on ACT in one op
            A.square(out=PRA[:, 6:9, qs, :], in_=xt[:, 1:4, qs, :])

        NSPLIT_T = int(_os.environ.get("KB_SPLIT_TILES", "4"))

        def emit_front_half(t, h):
            """DMA + products for q-half h of tile t (h=0 allocates the
            tile's buffers). Unsplit tiles emit everything at h=0."""
            if h == 0:
                xt = xpool.tile([128, D, NQ, 128], F16, tag="xt", name="xt")
                PRA = prpool.tile([128, 9, NQ, 128], F16, tag="PRA",
                                  name="PRA")
                fronts[t] = (xt, PRA)
            xt, PRA = fronts[t]
            xin = x_all[t].rearrange("p (d q b) -> p d q b", d=D, q=NQ)
            if t < NSPLIT_T:
                qs = slice(4 * h, 4 * h + 4)
            else:
                assert h == 0
                qs = slice(0, NQ)
            nc.sync.dma_start(out=xt[:, :, qs, :], in_=xin[:, :, qs, :])
            emit_products(t, xt, PRA, qs)

        def emit_front(t):
            emit_front_half(t, 0)
            if t < NSPLIT_T:
                emit_front_half(t, 1)

        # deferred product-stat copies for chunk-trigger tiles: the solve's
        # stage 1 needs only T-stats, so that block is copied early and the
        # rest is emitted from inside the solve generator (keeps the big
        # copy out of ACT's queue ahead of stage-1 ops)
        deferred_copy = {}

        def emit_stats(t):
            """104 reversed matmuls into PSUM, then copy to chunk ST."""
            ci = next(i for i, (t0, ct) in enumerate(CHUNKS)
                      if t0 <= t < t0 + ct)
            t0, ct = CHUNKS[ci]
            _tspl = _os.environ.get("KB_TSPLIT", "0")
            trigger = (t == t0 + ct - 1 and
                       (_tspl == "1" or
                        (_tspl == "last" and ci == len(CHUNKS) - 1)))
            xt, PRA = fronts.pop(t)
            PS = wpool.tile([128, NQ, NS, NG], F32, tag="PS", name="PS")
            if t == 0 and _os.environ.get("KB_T0Q", "0") == "1":
                halves = [range(0, 2), range(2, 5), range(5, 8)]
            elif t < NSPLIT_T:
                halves = [range(0, 4), range(4, 8)]
            else:
                halves = [range(NQ)]

            def mms(qr, stats):
                for q in qr:
                    for s in stats:
                        lhsT = (xt[:, s, q, :] if s < 4 else
                                PRA[:, STAT_RHS[s], q, :])
                        nc.tensor.matmul(PS[:, q, s, :], lhsT, W,
                                         start=True, stop=True)

            ST = STc[ci]
            if trigger:
                # T-stat matmuls for all q, early T-block copy, then the
                # product matmuls; the remaining copy is deferred into the
                # solve generator (after its T-only stage 1)
                for qr in halves:
                    mms(qr, [s for s in STAT_ORDER if s < 4])
                A.copy(out=ST[:, t - t0, :, 0:16],
                       in_=PS[:, :, 0:4, :].rearrange("p q s g -> p q (s g)"))
                for qr in halves:
                    mms(qr, [s for s in STAT_ORDER if s >= 4])
                def copy_rest():
                    A.copy(out=ST[:, t - t0, :, 16:52],
                           in_=PS[:, :, 4:13, :].rearrange(
                               "p q s g -> p q (s g)"))
                deferred_copy[ci] = copy_rest
            else:
                for qr in halves:
                    mms(qr, STAT_ORDER)
                A.copy(out=ST[:, t - t0],
                       in_=PS.rearrange("p q s g -> p q (s g)"))

        # per-chunk narrow-op engine cycle: Pool early (stream has DVE busy
        # with products), DVE late (post-stream DVE is idle)
        _engs = _os.environ.get("KB_ENG", "").split(",")

        def emit_solve(ci, qs=None):
            """Generator: yields between op groups so the driver can
            interleave solve emission with later tiles' work."""
            t0, ct = CHUNKS[ci]
            qlo, qhi = qs if qs else (0, NQ)
            qn = qhi - qlo
            ST = STc[ci][:, :, qlo:qhi, :]
            pat = (_engs[ci] if ci < len(_engs) and _engs[ci] else
                   ("G" if ci < len(CHUNKS) - 1 else "GV"))
            narrow_cycle = itertools.cycle(
                [{"G": G, "V": V}[ch] for ch in pat])
            _wengs = _os.environ.get("KB_WENG", "").split(",")
            wpat = (_wengs[ci] if ci < len(_wengs) and _wengs[ci] else "V")
            wide_cycle = itertools.cycle(
                [{"G": G, "V": V}[ch] for ch in wpat])

            def stat(s):
                return ST[:, :, :, 4 * s:4 * s + 4]

            def wide(lo, k):
                """k adjacent stats as [128, ct, qn, k, 4]."""
                return ST[:, :, :, 4 * lo:4 * (lo + k)].rearrange(
                    "p t q (k g) -> p t q k g", g=4)

            def bcast(v, n):
                """insert a stride-0 dim of size n before the last dim."""
                lay = [list(p) for p in v.ap]
                lay.insert(len(lay) - 1, [0, n])
                return BassAP(v.tensor, v.offset, lay)

            def bcast_after(v, n):
                """append a stride-0 dim of size n after the last dim."""
                lay = [list(p) for p in v.ap] + [[0, n]]
                return BassAP(v.tensor, v.offset, lay)

            def bcast_at(v, n, pos):
                """insert a stride-0 dim of size n at dim position pos."""
                lay = [list(p) for p in v.ap]
                lay.insert(pos, [0, n])
                return BassAP(v.tensor, v.offset, lay)

            def slotv(t9, start, step, n):
                """view slots (start, start+step, ...) of a k-slot tile."""
                lay = [list(p) for p in t9.ap]
                lay[3] = [4 * step, n]
                return BassAP(t9.tensor, t9.offset + 4 * start, lay)

            last = ci == len(CHUNKS) - 1
            d_, g_, i_ = stat(1), stat(2), stat(3)
            r3 = stat(0)
            DGI = wide(1, 3)               # (T1, T2, T3) = (d, g, i)
            DIAG = wide(4, 3)              # (S11, S22, S33)
            CROSS = wide(7, 3)             # (S12, S13, S23)
            RXI = wide(10, 3)              # (S01, S02, S03)

            def tmpw(name, k, dt=F16):
                shape = [128, ct, qn, 4] if k == 1 else [128, ct, qn, k, 4]
                name = f"{name}_c{ci}q{qlo}"
                return lpool.tile(shape, dt, tag=name, name=name)

            def op(kind, out, u, v, wide=False, term=False):
                # wide (multi-stat) ops mostly DVE (fp16 2x); narrow per
                # policy pattern; the last chunk's terminal chain stays on
                # DVE to avoid cross-engine hops on the kernel's end path
                _term = _os.environ.get("KB_TERM", "0")
                if term and last and _term in ("V", "G"):
                    eng = V if _term == "V" else G
                else:
                    eng = next(wide_cycle) if wide else next(narrow_cycle)
                getattr(eng, f"tensor_{kind}")(out=out, in0=u, in1=v)

            def nop(kind, name, k, u, v, dt=F16, term=False):
                t_ = tmpw(name, k, dt)
                op(kind, t_, u, v, wide=(k >= 2), term=term)
                return t_

            def sl(t_, a, b=None, step=1):
                """slots a..b (or single slot a) of a k-slot tile."""
                if b is None:
                    return t_[:, :, :, a, :]
                lay = [list(p) for p in t_.ap]
                lay[3] = [4 * step, b - a + 1 if step > 0 else a - b + 1]
                return BassAP(t_.tensor, t_.offset + 4 * a, lay)

            def rsl(t_, a, step, n):
                """n slots starting at a with slot stride `step` (can be <0,
                >1)."""
                lay = [list(p) for p in t_.ap]
                lay[3] = [4 * step, n]
                return BassAP(t_.tensor, t_.offset + 4 * a, lay)

            # solve-squares engine: ACT offloads DVE/Pool mid-stream; the
            # tail chunk can keep them on DVE (shorter chain) via KB_SOLVESQ
            sq_dve = last and _os.environ.get("KB_SOLVESQ", "A") == "V"

            def solve_sq(out, in_):
                if sq_dve:
                    V.tensor_mul(out=out, in0=in_, in1=in_)
                else:
                    A.square(out=out, in_=in_)

            # ---- Schur elimination of column 4 (pivot = 1 after scaling).
            # Schur matrix [[a b c],[b e f],[c f h]]: a=S11-d2 etc.
            # Pdiag (dd, gg, ii) on ACT (square); Pcross (dg, di, gi).
            Pdiag = tmpw("Pdiag", 3)
            solve_sq(Pdiag, DGI)
            Pcross = tmpw("Pcross", 3)
            op("mul", sl(Pcross, 0, 1), bcast(d_, 2), wide(2, 2), wide=True)
            op("mul", sl(Pcross, 2), g_, i_)
            CPp = nop("mul", "CPp", 3, bcast(r3, 3), DGI)
            dc = deferred_copy.pop(ci, None)
            if dc:
                dc()
            yield
            # SCH6 slots: [b, c, f, a, e, h]
            SCH = tmpw("SCH", 6)
            op("sub", sl(SCH, 0, 2), CROSS, Pcross, wide=True)
            op("sub", sl(SCH, 3, 5), DIAG, Pdiag, wide=True)
            # sign-flipped c (c' = r3*L - u) so z_i = n_i * rdet directly
            C3 = nop("sub", "C3", 3, CPp, RXI)               # c1', c2', c3'
            yield

            # ---- symmetric 3x3 adjugate products
            # CSQ = (b2, c2, f2) on ACT; EH2 = (eh, ah); ae;
            # PP1 = (cf, bf); PP2 = (bh, ce); PP3 = (cb, af)
            CSQ = tmpw("CSQ", 3)
            solve_sq(CSQ, sl(SCH, 0, 2))
            EH2 = nop("mul", "EH2", 2, rsl(SCH, 4, -1, 2), bcast(sl(SCH, 5), 2))
            ae = nop("mul", "ae", 1, sl(SCH, 3), sl(SCH, 4))
            PP1 = nop("mul", "PP1", 2, rsl(SCH, 1, -1, 2), bcast(sl(SCH, 2), 2))
            PP2 = nop("mul", "PP2", 2, sl(SCH, 0, 1), rsl(SCH, 5, -1, 2))
            PP3 = nop("mul", "PP3", 2, rsl(SCH, 1, 2, 2), rsl(SCH, 0, 2, 2))
            yield
            # ADJ row-major 9 slots; off-diagonals written to both mirrors
            ADJ = tmpw("ADJ", 9)
            op("sub", rsl(ADJ, 0, 4, 2), EH2, rsl(CSQ, 2, -1, 2), wide=True)
            op("sub", sl(ADJ, 8), ae, sl(CSQ, 0))            # A33 = ae - b2
            op("sub", sl(ADJ, 1, 2), PP1, PP2, wide=True)    # A12, A13
            op("sub", rsl(ADJ, 3, 3, 2), PP1, PP2, wide=True)  # mirrors
            op("sub", rsl(ADJ, 5, 2, 2), bcast(sl(PP3, 0), 2),
               bcast(sl(PP3, 1), 2), wide=True)              # A23 -> 5,7
            yield

            # det3 = a*A11 + (b, c) . (A12, A13)
            T3x = nop("mul", "T3x", 2, sl(SCH, 0, 1), sl(ADJ, 1, 2))
            aA11 = nop("mul", "aA11", 1, sl(SCH, 3), sl(ADJ, 0))
            dts = nop("add", "dts", 1, sl(T3x, 0), sl(T3x, 1))
            det3 = nop("add", "det3", 1, dts, aA11, F32)
            yield
            # all nine adj*c products in one op, then two strided-slice adds
            N9 = nop("mul", "N9", 9, ADJ, bcast_at(C3, 3, 3))
            T2 = nop("add", "T2", 3, slotv(N9, 0, 3, 3), slotv(N9, 1, 3, 3))
            N3 = nop("add", "N3", 3, T2, slotv(N9, 2, 3, 3))
            yield

            # dn = (d, g, i) . (n1, n2, n3)
            DN3 = nop("mul", "DN3", 3, DGI, N3)
            dns = nop("add", "dns", 1, sl(DN3, 0), sl(DN3, 1), term=True)
            dn = nop("add", "dn", 1, dns, sl(DN3, 2), term=True)
            yield

            rdet = tmpw("rdet", 1, F32)
            if _os.environ.get("KB_RECIP", "fast") == "fast":
                V.reciprocal_approx_fast(
                    out=rdet.rearrange("p t q g -> p (t q g)"),
                    in_=det3.rearrange("p t q g -> p (t q g)"),
                )
            else:
                scratch = tmpw("rscratch", 1, F32)
                V.reciprocal_approx_accurate(
                    out=rdet.rearrange("p t q g -> p (t q g)"),
                    in_=det3.rearrange("p t q g -> p (t q g)"),
                    scratch=scratch.rearrange("p t q g -> p (t q g)"),
                )
            yield

            OUT = lpool.tile([128, ct, 4 * qn, D], F32, tag=f"OUT{ci}q{qlo}",
                             name=f"OUT{ci}q{qlo}")
            OUT5 = OUT.rearrange("p t (q g) d -> p t q g d", q=qn)
            dnr = nop("mul", "dnr", 1, dn, rdet, term=True)
            # last chunk: assemble + DMA per ct-tile so the first tile's
            # output DMA overlaps the rest of the assembly
            tslices = ([slice(k, k + 1) for k in range(ct)]
                       if (last and ct > 1 and
                           _os.environ.get("KB_OSPLIT", "0") == "1")
                       else [slice(0, ct)])
            for tsl in tslices:
                # z_i = n_i * rdet in one op: transpose N3's (k, g) view to
                # match OUT's (g, comp) order, broadcast rdet over comps
                op("mul", OUT5[:, tsl, :, :, 0:3],
                   N3[:, tsl].rearrange("p t q k g -> p t q g k"),
                   bcast_after(rdet[:, tsl], 3), wide=True)
                # z4 = r3 + dn'*rdet  (det3*rdet == 1; n' carry the sign)
                op("add", OUT5[:, tsl, :, :, 3], r3[:, tsl], dnr[:, tsl],
                   term=True)
                # out-DMA on the ACT engine's HWDGE so it never head-of-line
                # blocks the SP queue carrying input DMAs; the last chunk
                # goes via SP (idle by then, slightly faster DGE path)
                _odma = (nc.sync.dma_start if last and
                         _os.environ.get("KB_LASTSP", "1") == "1"
                         else A.dma_start)
                _odma(
                    out=y_all[:, t0 + (tsl.start or 0):t0 + tsl.stop,
                              4 * qlo:4 * qhi],
                    in_=OUT[:, tsl])

        # Pumped emission: after each tile's stats, advance pending solve
        # generators round-robin by a bounded number of yield-groups, so
        # several chunks' dependency chains overlap.
        pending = []

        def pump(budget):
            i = 0
            while budget > 0 and pending:
                try:
                    next(pending[i % len(pending)])
                    i += 1
                    budget -= 1
                except StopIteration:
                    pending.pop(i % len(pending))

        ready = {t0 + ct - 1: ci for ci, (t0, ct) in enumerate(CHUNKS)}
        # chunks whose solve runs as two parallel q-half chains (late chunks:
        # post-stream, both DVE and Pool are idle, so two chains interleave)
        nsplit = int(_os.environ.get("KB_QSPLIT", "0"))
        split = set(range(max(0, len(CHUNKS) - nsplit), len(CHUNKS)))
        emit_front(0)
        emit_front(1)
        for t in range(NT):
            if t + 2 < NT:
                emit_front(t + 2)
            emit_stats(t)
            if t in ready:
                ci = ready[t]
                if ci in split:
                    pending.append(emit_solve(ci, qs=(0, 4)))
                    pending.append(emit_solve(ci, qs=(4, NQ)))
                else:
                    pending.append(emit_solve(ci))
            pump(PUMP_GROUPS.get(t, 0))
        while pending:
            pump(1 << 30)


_NC_CACHE = {}


def _get_nc():
    if "nc" not in _NC_CACHE:
        nc = bacc.Bacc("TRN2", target_bir_lowering=False, debug=False,
                       num_devices=NCORES)
        xd = nc.dram_tensor("x", [NT, 128, D * NQ * 128], F16,
                            kind="ExternalInput")
        yd = nc.dram_tensor("y", [BC, D], F32, kind="ExternalOutput")
        with tile.TileContext(nc) as tc:
            _emit(nc, tc, xd, yd)
        nc.compile()
        _NC_CACHE["nc"] = nc
    return _NC_CACHE["nc"]


def _stage(xk):
    """[BC, M, D] fp32 -> [NT, 128, 4096] fp16 fall layout."""
    xr = xk.reshape(NT, 128, NQ, NG, M, D)       # t p q g m d
    xs = xr.transpose(0, 3, 4, 5, 2, 1)          # t g m d q p
    return np.ascontiguousarray(xs.astype(np.float16)).reshape(
        NT, 128, D * NQ * 128)


def run_sharded(x, trace=False, **kwargs):
    nc = _get_nc()
    in_maps = [
        {"x": _stage(x[k * BC:(k + 1) * BC])}
        for k in range(NCORES)
    ]
    res = run_bass_kernel_spmd(nc, in_maps, core_ids=list(range(NCORES)),
                               trace=trace, **kwargs)
    out = np.concatenate([res.results[k]["y"] for k in range(NCORES)], axis=0)
    return out, res


def kernel(**inputs):
    x = np.asarray(inputs["x"], dtype=np.float32)
    out, _ = run_sharded(x)
    return out
